# revision 10
# baseline (speedup 1.0000x reference)
"""CLAHE/LCN kernel for Trainium2, 8-core data parallel. v4.

Math (per image, 31x31 'same' zero-padded box window):
    S  = box2d(x)   (sum)      Q = box2d(x^2)   (sum)
    mean = S/961, var = Q/961 - mean^2, std = sqrt(var)
    out  = 0.2*x + 0.8*sigmoid(0.5*(x-mean)/std)

Box filter on PE: image block X_b as stationary lhsT [K=128, M=128]
against a banded 0/1 moving operand computes the column 31-box of X
transposed; two such fused transpose+box stages give the full 2D box
back in natural layout with no transposes.

Host protocol: host sends xh = fp16(x)*0.25, kernel returns
u = xh + sigmoid(norm/2) in fp16, host multiplies by 0.8 during the
f32 upcast: 0.8*u = 0.2x + 0.8*sigmoid = reference output. The final
affine therefore costs zero device ops.

Engine split (DVE is the wall — every op counted):
  DVE:  tb = xh^2 (2x TT), t1t evac + 1 t1x tile (1x PSUM copies),
        fused var = 16c*Q - 16*Sb^2 (custom op, reads Q psum),
        num = xh - Sb (2x TT), z = num*rc (2x TT).
  ACT:  7 t1x evac tiles, Sb = c*S psum copies (scale folded),
        rc = 1/sqrt(var) (Abs_reciprocal_sqrt LUT), sigmoid LUT.
        Table sets pinned to exactly {AbsRsqrt,Copy} / {Sigmoid} so the
        set switches exactly twice per image.
  SWDGE/DMA: u = xh += sigmoid via accumulate-DMA (CCE add) in place,
        then DMA-out straight from the xh tile.
  PE:   all 4 banded-MM passes.

Software pipeline with a one-image lag: alpha(i) [DMA, tb, stage1+evac,
stage2+Sb+var] is emitted before beta(i-1) [num, rc, z, sigmoid,
accum, out-DMA] so image i's PSUM evacs never queue behind image
i-1's tail (head-of-line blocking was the v2 stall).
"""

import threading

import numpy as np

# ---------------------------------------------------------------- constants
B_FULL = 32          # full batch
NCORES = 8
IMGS = B_FULL // NCORES  # images per core
H = W = 1024
P = 128              # partitions
NBLK = H // P        # 8 row blocks per image
NH = 2               # halves per image (4 row-tiles each)
KWIN = 31
HALF = KWIN // 2     # 15
AREA_INV = 1.0 / (KWIN * KWIN)  # 1/961

_lock = threading.Lock()
_compiled = None  # (nc, band_np)


def _band_spec():
    """Per h-block b: (lo, hi, offset into packed band array)."""
    spec = []
    off = 0
    for b in range(NBLK):
        lo = max(0, P * b - HALF)
        hi = min(H, P * b + P + HALF + 1)  # 128b+143
        spec.append((lo, hi, off))
        off += hi - lo
    return spec, off


def _band_np():
    spec, total = _band_spec()
    band = np.zeros((P, total), np.float32)
    for b, (lo, hi, off) in enumerate(spec):
        for h in range(P):
            gh = P * b + h
            r0 = max(lo, gh - HALF)
            r1 = min(hi, gh + HALF + 1)
            band[h, off + (r0 - lo): off + (r1 - lo)] = 1.0
    return band.astype(np.float16)


def _mm_segments():
    """Matmul segments for one output tile [128, 1024]: (b, s0, s1,
    band_off, start, stop), clipped to PSUM bank boundaries (512 fp32);
    start/stop mark the first/last MM touching each bank."""
    spec, _ = _band_spec()
    per_block = []
    for b, (lo, hi, off) in enumerate(spec):
        for bank in (0, 1):
            s0 = max(lo, 512 * bank)
            s1 = min(hi, 512 * bank + 512)
            if s1 > s0:
                per_block.append((b, s0, s1, off + (s0 - lo), bank))
    out = []
    seen = set()
    last_idx = {}
    for i, (b, s0, s1, boff, bank) in enumerate(per_block):
        last_idx[bank] = i
    for i, (b, s0, s1, boff, bank) in enumerate(per_block):
        start = bank not in seen
        seen.add(bank)
        out.append((b, s0, s1, boff, start, last_idx[bank] == i))
    return out


def _register_var_op():
    """Custom DVE op: out = s0*in0 + s1*(in1*in1).
    With in0 = Q psum (f32), in1 = Sb = mean/4 (fp16), s0 = 16/961,
    s1 = -16.0 this computes var = E[x^2] - mean^2 in ONE pass, fusing
    the mean^2 square."""
    import concourse.dve_ops as dve_ops
    from concourse.dve_spec import Spec, Src0, Src1, C0, C1, sq, lower
    from concourse.dve_spec import _has_src1
    from concourse.dve_uop import DveOpSpec

    name = "VAR_CLAHE"
    for op in dve_ops.OPS:
        if op.name == name:
            return op
    spec = Spec(
        body=Src0 * C0 + sq(Src1) * C1,
        reference=lambda in0, in1, s0, s1, imm2: (
            in0.astype(np.float32) * s0
            + np.square(in1.astype(np.float32)) * s1),
    )
    row = dve_ops._CUSTOM_DVE_ROW_BASE + len(dve_ops.OPS)
    shas = {}
    for ver in ("v3",):
        uops = lower(spec, ver=ver)
        shas[ver] = DveOpSpec(name=name, opcode=row, uops=uops,
                              rd1_en=_has_src1(spec)).sha(ver)
    op = dve_ops.DveOp(name, spec, subdim=False, uops_sha=shas)
    dve_ops.OPS.append(op)
    dve_ops._SUB_OPCODE_FOR_NAME[name] = row
    dve_ops.CUSTOM_DVE_SPECS[name] = op.spec
    return op


def _patch_act_tables():
    """Pin the ACT table sets: one set holds exactly
    {Abs_reciprocal_sqrt, Copy}, another exactly {Sigmoid}; every other
    set is hollowed.  This forces the table-load inserter to a unique
    assignment (v3 measured 4 loads/image from a looser patch) so the
    set switches exactly twice per image. Dict order (set IDs) is
    unchanged so the emitted IDs stay valid."""
    import concourse.bacc as bacc_mod
    from concourse import mybir
    if getattr(bacc_mod, "_clahe_tables_patched", False):
        return
    orig = bacc_mod.get_activation_tables
    A = mybir.ActivationFunctionType

    def patched(arch):
        tabs = dict(orig(arch))
        for k in tabs:
            if k == "abs_reciprocal_sqrt_and_small":
                pass  # keep full contents: Copy maps uniquely here
            elif k == "sigmoid_and_others":
                tabs[k] = tabs[k] & {A.Sigmoid}
            else:
                tabs[k] = set()
        return tabs

    bacc_mod.get_activation_tables = patched
    bacc_mod._clahe_tables_patched = True


def _build():
    import concourse.bacc as bacc
    import concourse.tile as tile
    from concourse import mybir

    _patch_act_tables()
    var_op = _register_var_op()

    f16 = mybir.dt.float16
    f32 = mybir.dt.float32
    ALU = mybir.AluOpType
    ACT = mybir.ActivationFunctionType

    mm_segs = _mm_segments()
    _, band_w = _band_spec()
    c = AREA_INV

    nc = bacc.Bacc("TRN2", target_bir_lowering=False, debug=False,
                   num_devices=NCORES)
    x_ext = nc.dram_tensor("x", [IMGS * H, W], f16, kind="ExternalInput")
    band_ext = nc.dram_tensor("band", [P, band_w], f16, kind="ExternalInput")
    y_ext = nc.dram_tensor("y", [IMGS * H, W], f16, kind="ExternalOutput")
    x_ap = x_ext.ap()
    y_ap = y_ext.ap()

    with tile.TileContext(nc) as tc:
        from contextlib import ExitStack
        with ExitStack() as ctx:
            def pool(name, bufs):
                return ctx.enter_context(tc.tile_pool(name=name, bufs=bufs))

            singles = pool("singles", 1)
            p_xh = pool("p_xh", 2)     # xh full image [P,8,W] fp16
            p_tb = pool("p_tb", 1)     # xh^2 full image [P,8,W] fp16
            p_t1 = pool("p_t1", 1)     # t1x/t1t [P,8,W] fp16 (2 tags)
            p_sb = pool("p_sb", 4)     # Sb = mean/4 halves [P,4,W] fp16
            p_v = pool("p_v", 4)       # var halves [P,4,W] fp16
            p_num = pool("p_num", 2)   # num halves [P,4,W] fp16 (u reuses)
            p_rcp = pool("p_rcp", 2)   # 1/std halves [P,4,W] fp16
            p_z = pool("p_z", 2)       # z halves [P,4,W] fp16
            p_thu = pool("p_thu", 1)   # sigmoid halves [P,4,W] fp16
            ps_1 = ctx.enter_context(
                tc.tile_pool(name="ps1", bufs=2, space="PSUM"))
            ps_s = ctx.enter_context(
                tc.tile_pool(name="psS", bufs=1, space="PSUM"))
            ps_q = ctx.enter_context(
                tc.tile_pool(name="psQ", bufs=1, space="PSUM"))

            band_sb = singles.tile([P, band_w], f16)
            nc.sync.dma_start(out=band_sb[:], in_=band_ext.ap())

            def stage_mms(ps, stat_slicer):
                """Banded MM group for one [128,1024] output tile into a
                two-bank PSUM tile ps [P, 1024]."""
                for (b, s0, s1, boff, first, last) in mm_segs:
                    nc.tensor.matmul(
                        ps[:, s0:s1],
                        stat_slicer(b),
                        band_sb[:, boff: boff + (s1 - s0)],
                        start=first, stop=last,
                    )

            def emit_alpha(img):
                """DMA-in, tb, stage-1 (+evacs), stage-2 (+Sb, fused
                var). Returns state consumed by emit_beta one image
                later."""
                base = img * H
                xh = p_xh.tile([P, NBLK, W], f16, tag="xh")
                tb = p_tb.tile([P, NBLK, W], f16, tag="tb")
                for q in range(4):
                    nc.sync.dma_start(
                        out=xh[:, 2 * q: 2 * q + 2, :],
                        in_=y_rows(x_ap, base + 256 * q, 2))
                for h in range(NH):
                    sl = (slice(None), slice(4 * h, 4 * h + 4), slice(None))
                    nc.vector.tensor_mul(tb[sl], xh[sl], xh[sl])

                # stage 1: fused transpose+colbox for x and x^2
                t1x = p_t1.tile([P, NBLK, W], f16, tag="t1x")
                t1t = p_t1.tile([P, NBLK, W], f16, tag="t1t")
                for wt in range(NBLK):
                    ps = ps_1.tile([P, 2 * 512], f32, tag="ps1")
                    stage_mms(ps, lambda b: xh[:, b, wt * P:(wt + 1) * P])
                    if wt < 7:
                        nc.scalar.copy(out=t1x[:, wt, :], in_=ps[:])
                    else:
                        nc.vector.tensor_copy(t1x[:, wt, :], ps[:])
                for wt in range(NBLK):
                    ps = ps_1.tile([P, 2 * 512], f32, tag="ps1")
                    stage_mms(ps, lambda b: tb[:, b, wt * P:(wt + 1) * P])
                    nc.vector.tensor_copy(t1t[:, wt, :], ps[:])

                # stage 2: S and Q per row-tile; Sb + fused var
                sb_h = []
                vb_h = []
                for h in range(NH):
                    sb = p_sb.tile([P, 4, W], f16, tag="sb")
                    vb = p_v.tile([P, 4, W], f16, tag="vb")
                    for j in range(4):
                        m = 4 * h + j
                        ps_S = ps_s.tile([P, W], f32, tag="psS")
                        stage_mms(ps_S,
                                  lambda b: t1x[:, b, m * P:(m + 1) * P])
                        # Sb = c*S = mean/4   (fp16)
                        nc.scalar.activation(sb[:, j, :], ps_S[:], ACT.Copy,
                                             bias=0.0, scale=c)
                        ps_Q = ps_q.tile([P, W], f32, tag="psQ")
                        stage_mms(ps_Q,
                                  lambda b: t1t[:, b, m * P:(m + 1) * P])
                        # var = 16c*Q - 16*Sb^2   (one fused DVE op)
                        nc.vector._custom_dve(
                            var_op, out=vb[:, j, :],
                            in0=ps_Q[:], in1=sb[:, j, :],
                            s0=16.0 * c, s1=-16.0)
                    sb_h.append(sb)
                    vb_h.append(vb)
                return dict(base=base, xh=xh, sb_h=sb_h, vb_h=vb_h)

            def emit_beta(st):
                """num/rsqrt/z/sigmoid, then u = xh += sigmoid via
                accumulate-DMA (SWDGE CCE add) and DMA-out straight
                from the xh tile. Emitted AFTER the next image's alpha
                so that image's PSUM evacs don't queue behind this
                tail."""
                base, xh = st["base"], st["xh"]
                z_h = []
                for h in range(NH):
                    sl = (slice(None), slice(4 * h, 4 * h + 4), slice(None))
                    # num = xh - Sb = 0.25*(x - mean)   (fp16 TT -> 2x)
                    nb = p_num.tile([P, 4, W], f16, tag="numq")
                    nc.vector.tensor_sub(nb[:], xh[sl], st["sb_h"][h][:])
                    # rc = 1/sqrt(var) = 1/std   (fp16)
                    rc = p_rcp.tile([P, 4, W], f16, tag="rcp")
                    nc.scalar.activation(rc[:], st["vb_h"][h][:],
                                         ACT.Abs_reciprocal_sqrt,
                                         bias=0.0, scale=1.0)
                    # z = num * rc = 0.25*norm   (fp16 TT -> 2x)
                    zt = p_z.tile([P, 4, W], f16, tag="z")
                    nc.vector.tensor_mul(zt[:], nb[:], rc[:])
                    z_h.append(zt)
                for h in range(NH):
                    sl = (slice(None), slice(4 * h, 4 * h + 4), slice(None))
                    # sigmoid(2z) = sigmoid(norm/2)   (fp16)
                    th = p_thu.tile([P, 4, W], f16, tag="thu")
                    nc.scalar.activation(th[:], z_h[h][:], ACT.Sigmoid,
                                         bias=0.0, scale=2.0)
                    # u = xh + sigmoid (fp16 TT -> 2x); host multiplies
                    # by 0.8: 0.8*u = 0.2x + 0.8*sigmoid. Reuses a num
                    # buffer (dead after z).
                    ut = p_num.tile([P, 4, W], f16, tag="numq")
                    nc.vector.tensor_add(ut[:], xh[sl], th[:])
                    nc.sync.dma_start(
                        out=y_rows(y_ap, base + 512 * h, 4), in_=ut[:])

            # Software pipeline with a one-image lag between the
            # PE-heavy alpha phase and the DVE/ACT-heavy beta tail.
            prev = None
            for img in range(IMGS):
                st = emit_alpha(img)
                if prev is not None:
                    emit_beta(prev)
                prev = st
            emit_beta(prev)

    nc.compile()
    return nc


def y_rows(dram_ap, row0, t):
    """DRAM AP view [P, t, W]: element (p, i, c) <-> dram[row0+128i+p, c]."""
    sl = dram_ap[row0: row0 + 128 * t, :]
    return sl.rearrange("(t p) c -> p t c", p=P)


def _get_compiled():
    global _compiled
    with _lock:
        if _compiled is None:
            band = np.ascontiguousarray(_band_np())
            nc = _build()
            _compiled = (nc, band)
    return _compiled


def _run(x, trace=False, **kw):
    from concourse.bass_utils import run_bass_kernel_spmd

    nc, band = _get_compiled()
    x = np.asarray(x, dtype=np.float32).reshape(B_FULL, H, W)
    # fp16(x) * 0.25 is exact (exponent shift): xh = 0.25*x in fp16.
    xh = x.astype(np.float16) * np.float16(0.25)
    core_ids = list(range(NCORES))
    in_maps = []
    for i in core_ids:
        xs = np.ascontiguousarray(
            xh[IMGS * i: IMGS * (i + 1)].reshape(IMGS * H, W))
        in_maps.append({"x": xs, "band": band})
    res = run_bass_kernel_spmd(nc, in_maps, core_ids, trace=trace, **kw)
    # Device returns u = 0.25x + sigmoid; 0.8*u is the reference output.
    out = np.concatenate(
        [(res.results[i]["y"].astype(np.float32) * 0.8)
         .reshape(IMGS, 1, H, W) for i in core_ids], axis=0)
    return out, res


def kernel(x):
    out, _ = _run(x, trace=False)
    return out
